# revision 2
# baseline (speedup 1.0000x reference)
"""Multi-head attention block (QKV proj + RMSNorm + RoPE + SDPA + out proj)
sharded across 8 Trainium2 NeuronCores — v4.

Sharding: data-parallel over batch (B=2 -> 2 groups of 4 cores), tensor-parallel
over heads (16 heads -> 4 heads/core).  Each core computes a partial output
projection for its 4 heads (bf16); the host sums the 4 partials per batch and
adds bproj.

v4 design vs v3:
  - exp reads scores PSUM directly and writes the bf16 stage to SBUF (no DVE
    PSUM->SBUF staging CASTs, which were ~200us of DVE busy in v3).
  - The attention inner loop alternates the two q-halves of each k-chunk
    between the ACT engine (true exp) and the DVE engine (Schraudolph exp:
    bf16-bits = int16(t*184.665 + 16250.5), written through a bitcast) so the
    PE never waits on exp.  PE gaps reset the tensor engine to its mid p-state
    (1.2 GHz); keeping it dense holds 2.4 GHz.
  - k-side RMSNorm rsqrt (and the 1/sqrt(hd) scale) are folded into the exp's
    per-partition scale operand instead of being broadcast-multiplied into kT.
  - Scores matmuls hold one k-chunk stationary for all 2048 q columns;
    projection loops hold each weight chunk stationary for 4x512 moving cols.
  - Softmax normalization runs on GPSIMD (partition_broadcast) + DVE, off PE.
  - Output partials are written bf16 (halves the output DMA).
"""

import contextlib

import numpy as np
import ml_dtypes

B, S, D, H = 2, 2048, 1024, 16
HD = D // H
N_CORES = 8
HPC = H // 4  # heads per core = 4
CW = HPC * HD  # per-core head-col width = 256

BF16 = ml_dtypes.bfloat16

# Schraudolph exp constants for bf16-bits: bits16 = t*ALPHA + BETA
SCH_ALPHA = 184.6650292
SCH_BETA = 16250.5

LAST_RESULTS = None


def _build_bass(reps=1, sim=False):
    import concourse.mybir as mybir
    import concourse.tile as tile
    from concourse import bacc

    fp32 = mybir.dt.float32
    bf16 = mybir.dt.bfloat16
    int16 = mybir.dt.int16
    AF = mybir.ActivationFunctionType
    Alu = mybir.AluOpType

    if sim:
        nc = bacc.Bacc(None, target_bir_lowering=False, debug=True)
    else:
        nc = bacc.Bacc()

    # ---- DRAM I/O ----
    xT = nc.dram_tensor("xT", [D, S], bf16, kind="ExternalInput")
    wqk = nc.dram_tensor("wqk", [D, 2 * CW], bf16, kind="ExternalInput")
    wv = nc.dram_tensor("wv", [D, CW], bf16, kind="ExternalInput")
    wpr = nc.dram_tensor("wpr", [CW, D], bf16, kind="ExternalInput")
    cosT2 = nc.dram_tensor("cosT2", [128, S], bf16, kind="ExternalInput")
    sinT2 = nc.dram_tensor("sinT2", [128, S], bf16, kind="ExternalInput")
    perm = nc.dram_tensor("perm", [128, 128], bf16, kind="ExternalInput")
    mask97 = nc.dram_tensor("mask97", [128, 2 * 97], bf16, kind="ExternalInput")
    sel97 = nc.dram_tensor("sel97", [97, 2 * 128], bf16, kind="ExternalInput")
    bqk = nc.dram_tensor("bqk", [128, 4], fp32, kind="ExternalInput")
    bvb = nc.dram_tensor("bvb", [128, CW], bf16, kind="ExternalInput")
    out = nc.dram_tensor("out", [S, D], bf16, kind="ExternalOutput")

    with tile.TileContext(nc) as tc:
        with tc.tile_pool(name="persist", bufs=1) as pp:
            xT_sb = pp.tile([128, 8, S], bf16, name="xT_sb")
            wqk_sb = pp.tile([128, 8, 2 * CW], bf16, name="wqk_sb")
            wv_sb = pp.tile([128, 8, CW], bf16, name="wv_sb")
            wpr_sb = pp.tile([128, 2, D], bf16, name="wpr_sb")
            cos_sb = pp.tile([128, S], bf16, name="cos_sb")
            sin_sb = pp.tile([128, S], bf16, name="sin_sb")
            perm_sb = pp.tile([128, 128], bf16, name="perm_sb")
            mask_sb = pp.tile([128, 2, 97], bf16, name="mask_sb")
            sel_sb = pp.tile([97, 2, 128], bf16, name="sel_sb")
            bqk_sb = pp.tile([128, 4], fp32, name="bqk_sb")
            bvb_sb = pp.tile([128, CW], bf16, name="bvb_sb")
            eb_sb = pp.tile([97, 1], fp32, name="eb_sb")
            qkT_sb = pp.tile([128, 4, S], bf16, name="qkT_sb")
            v_sb = pp.tile([128, 16, 4, 66], bf16, name="v_sb")
            oT_sb = pp.tile([128, 2, S], bf16, name="oT_sb")
            cq_sb = pp.tile([97, 2, S], bf16, name="cq_sb")
            rd_sb = pp.tile([1, S], fp32, name="rd_sb")

            rep_stack = contextlib.ExitStack()
            if reps > 1:
                rep_stack.enter_context(tc.For_i(0, reps))

            # ---- input DMAs ----
            nc.sync.dma_start(wv_sb[:], wv.rearrange("(c p) m -> p c m", p=128))
            for kk in range(8):
                nc.sync.dma_start(xT_sb[:, kk, :], xT[kk * 128 : (kk + 1) * 128, :])
            nc.sync.dma_start(wqk_sb[:], wqk.rearrange("(c p) m -> p c m", p=128))
            nc.sync.dma_start(cos_sb[:], cosT2[:])
            nc.sync.dma_start(sin_sb[:], sinT2[:])
            nc.sync.dma_start(perm_sb[:], perm[:])
            nc.sync.dma_start(mask_sb[:], mask97.rearrange("p (i m) -> p i m", i=2))
            nc.sync.dma_start(sel_sb[:], sel97.rearrange("p (i m) -> p i m", i=2))
            nc.sync.dma_start(bqk_sb[:], bqk[:])
            nc.sync.dma_start(bvb_sb[:], bvb[:])
            nc.sync.dma_start(wpr_sb[:], wpr.rearrange("(c p) m -> p c m", p=128))
            # ones column for the softmax denominator rider (col 64 of each head)
            nc.vector.memset(v_sb[:, :, :, 64:65], 1.0)
            # Exp bias: rows 0:64 (q) -> 0, rows 64:97 (k) -> ln(0.125) so the
            # k-side cq comes out pre-multiplied by 1/sqrt(HD).
            nc.vector.memset(eb_sb[0:64, :], 0.0)
            nc.vector.memset(eb_sb[64:97, :], float(np.log(0.125)))

            # ---------- Phase B1: V projection (v natural [kpos, chan]) ----
            with (
                tc.tile_pool(name="vps", bufs=3, space="PSUM") as vps,
            ):
                for si in range(16):
                    ps = vps.tile([128, CW], fp32, tag="vmm")
                    for kk in range(8):
                        nc.tensor.matmul(
                            ps[:],
                            xT_sb[:, kk, si * 128 : (si + 1) * 128],
                            wv_sb[:, kk, :],
                            start=(kk == 0),
                            stop=(kk == 7),
                        )
                    nc.vector.tensor_add(
                        out=v_sb[:, si, :, 0:64],
                        in0=ps[:].rearrange("p (h c) -> p h c", h=4),
                        in1=bvb_sb[:].rearrange("p (h c) -> p h c", h=4),
                    )

            # ---------- Phase B2+B3: QK projection + rmsnorm stats + rope ----
            with (
                tc.tile_pool(name="mmps", bufs=2, space="PSUM") as mmps,
                tc.tile_pool(name="ssps", bufs=1, space="PSUM") as ssps,
                tc.tile_pool(name="sqpool", bufs=1) as sqpool,
                tc.tile_pool(name="lnpool", bufs=1) as lnpool,
            ):
                sq_tiles = {}
                for g in range(2):
                    for mi, m in enumerate((g, 2 + g)):
                        # two [128,1024] psum tiles per m (segs 0-1 and 2-3)
                        pA = mmps.tile([128, 1024], fp32, tag="qkmm")
                        pB = mmps.tile([128, 1024], fp32, tag="qkmm")
                        for kk in range(8):
                            st = wqk_sb[:, kk, m * 128 : (m + 1) * 128]
                            for half, pt in ((0, pA), (1, pB)):
                                for s2 in range(2):
                                    nc.tensor.matmul(
                                        pt[:, s2 * 512 : (s2 + 1) * 512],
                                        st,
                                        xT_sb[
                                            :,
                                            kk,
                                            half * 1024
                                            + s2 * 512 : half * 1024
                                            + (s2 + 1) * 512,
                                        ],
                                        start=(kk == 0),
                                        stop=(kk == 7),
                                    )
                        for half, pt in ((0, pA), (1, pB)):
                            nc.vector.tensor_scalar_add(
                                out=qkT_sb[:, m, half * 1024 : (half + 1) * 1024],
                                in0=pt[:],
                                scalar1=bqk_sb[:, m : m + 1],
                            )
                        sq = sqpool.tile([128, S], bf16, tag=f"sq{mi}")
                        sq_tiles[m] = sq
                        qs = qkT_sb[:, m, :]
                        nc.vector.tensor_mul(out=sq[:], in0=qs, in1=qs)

                    # rmsnorm stats: ss[c, s] = sum_p mask[p,c] * sq[p,s]
                    ss = ssps.tile([97, 4, 512], fp32, tag="ss")
                    for mi, m in enumerate((g, 2 + g)):
                        for seg in range(4):
                            nc.tensor.matmul(
                                ss[:, seg, :],
                                mask_sb[:, mi, :],
                                sq_tiles[m][:, seg * 512 : (seg + 1) * 512],
                                start=(mi == 0),
                                stop=(mi == 1),
                            )
                    ln = lnpool.tile([97, S], fp32, tag="ln")
                    nc.scalar.activation(
                        ln[:],
                        ss[:].rearrange("p a b -> p (a b)"),
                        AF.Ln,
                        scale=1.0 / HD,
                    )
                    nc.scalar.activation(
                        cq_sb[:, g, :], ln[:], AF.Exp, scale=-0.5, bias=eb_sb[:, 0:1]
                    )

            # ---------- Phase B3: rope + q-side rms scale ----------
            with (
                tc.tile_pool(name="ropeps", bufs=2, space="PSUM") as ropeps,
                tc.tile_pool(name="ropetmp", bufs=2) as ropetmp,
            ):
                for g in range(2):
                    for m in (g, 2 + g):
                        for ch in range(2):
                            c0 = ch * 1024
                            qs_ps = ropeps.tile([128, 1024], fp32, tag="rps")
                            for seg in range(2):
                                nc.tensor.matmul(
                                    qs_ps[:, seg * 512 : (seg + 1) * 512],
                                    perm_sb[:],
                                    qkT_sb[:, m, c0 + seg * 512 : c0 + (seg + 1) * 512],
                                    start=True,
                                    stop=True,
                                )
                            t1 = ropetmp.tile([128, 1024], bf16, tag="t1")
                            nc.vector.tensor_mul(
                                out=t1[:],
                                in0=qkT_sb[:, m, c0 : c0 + 1024],
                                in1=cos_sb[:, c0 : c0 + 1024],
                            )
                            t2 = ropetmp.tile([128, 1024], bf16, tag="t2")
                            nc.vector.tensor_mul(
                                out=t2[:], in0=qs_ps[:], in1=sin_sb[:, c0 : c0 + 1024]
                            )
                            nc.vector.tensor_add(
                                out=qkT_sb[:, m, c0 : c0 + 1024], in0=t1[:], in1=t2[:]
                            )
                            # per-position rsqrt via selection matmul
                            # (k side comes pre-scaled by 1/sqrt(HD) via eb)
                            patt = 0 if m < 2 else 1
                            cq_ps = ropeps.tile([128, 1024], fp32, tag="cqb")
                            for seg in range(2):
                                nc.tensor.matmul(
                                    cq_ps[:, seg * 512 : (seg + 1) * 512],
                                    sel_sb[:, patt, :],
                                    cq_sb[
                                        :, g, c0 + seg * 512 : c0 + (seg + 1) * 512
                                    ],
                                    start=True,
                                    stop=True,
                                )
                            nc.vector.tensor_mul(
                                out=qkT_sb[:, m, c0 : c0 + 1024],
                                in0=qkT_sb[:, m, c0 : c0 + 1024],
                                in1=cq_ps[:],
                            )

            # ---------- Phase E: attention ----------
            # Per (p, h): j-loop over 16 k-chunks, q-halves split ACT/DVE for
            # exp.  AV lags scores by LAG chunks (deep stage ring) so exp
            # jitter never stalls the PE (PE gaps reset its p-state), and the
            # next iteration's scores sprint covers the normalize tail.
            LAG = 4
            with (
                tc.tile_pool(name="scps", bufs=1, space="PSUM") as scps,
                tc.tile_pool(name="otps", bufs=1, space="PSUM") as otps,
                tc.tile_pool(name="expool", bufs=LAG + 2) as expool,
                tc.tile_pool(name="dnpool", bufs=2) as dnpool,
                tc.tile_pool(name="rbpool", bufs=2) as rbpool,
            ):
                for p in range(2):
                    for h in range(2):
                        hh = 2 * p + h
                        pr = 64 * h
                        oT = otps.tile([65, S], fp32, tag="ot", name="ot")
                        stages = {}

                        def av(jp):
                            pa, pb = stages.pop(jp)
                            for qh, stt in ((0, pa), (1, pb)):
                                for s2 in range(2):
                                    nc.tensor.matmul(
                                        oT[
                                            :,
                                            qh * 1024
                                            + s2 * 512 : qh * 1024
                                            + (s2 + 1) * 512,
                                        ],
                                        v_sb[:, jp, hh, 0:65],
                                        stt[:, s2 * 512 : (s2 + 1) * 512],
                                        start=(jp == 0),
                                        stop=(jp == 15),
                                    )

                        for j in range(16):
                            sca = scps.tile([128, 1024], fp32, tag="sca", name="sca")
                            scb = scps.tile([128, 1024], fp32, tag="scb", name="scb")
                            kstat = qkT_sb[pr : pr + 64, 2 + p, j * 128 : (j + 1) * 128]
                            for qh, sct in ((0, sca), (1, scb)):
                                for s2 in range(2):
                                    nc.tensor.matmul(
                                        sct[:, s2 * 512 : (s2 + 1) * 512],
                                        kstat,
                                        qkT_sb[
                                            pr : pr + 64,
                                            p,
                                            qh * 1024
                                            + s2 * 512 : qh * 1024
                                            + (s2 + 1) * 512,
                                        ],
                                        start=True,
                                        stop=True,
                                    )
                            sta = expool.tile([128, 1024], bf16, tag="sta", name="sta")
                            stb = expool.tile([128, 1024], bf16, tag="stb", name="stb")
                            nc.scalar.activation(sta[:], sca[:], AF.Exp)
                            nc.vector.tensor_scalar(
                                out=stb[:].bitcast(int16),
                                in0=scb[:],
                                scalar1=float(SCH_ALPHA),
                                scalar2=float(SCH_BETA),
                                op0=Alu.mult,
                                op1=Alu.add,
                            )
                            stages[j] = (sta, stb)
                            if j >= LAG:
                                av(j - LAG)
                        for jp in range(16 - LAG, 16):
                            av(jp)

                        # normalize: oT_sb rows = oT[0:64] / denominator row
                        # (the bit-trick reciprocal cannot read PSUM on HW)
                        dn = dnpool.tile([1, S], fp32, tag="dn")
                        nc.scalar.activation(dn[:], oT[64:65, :], AF.Copy)
                        nc.vector.reciprocal_approx_fast(out=rd_sb[:], in_=dn[:])
                        rb = rbpool.tile([64, S], fp32, tag="rb")
                        nc.gpsimd.partition_broadcast(rb[:], rd_sb[0:1, :], channels=64)
                        nc.vector.tensor_mul(
                            out=oT_sb[pr : pr + 64, p, :],
                            in0=oT[0:64, :],
                            in1=rb[:],
                        )

            # ---------- Phase F: output projection (bf16 partials) ----------
            with (
                tc.tile_pool(name="prps", bufs=2, space="PSUM") as prps,
                tc.tile_pool(name="outpool", bufs=3) as outpool,
            ):
                for si in range(16):
                    ps = prps.tile([128, 1024], fp32, tag="pr")
                    for kc in range(2):
                        st = oT_sb[:, kc, si * 128 : (si + 1) * 128]
                        for ncol in range(2):
                            nc.tensor.matmul(
                                ps[:, ncol * 512 : (ncol + 1) * 512],
                                st,
                                wpr_sb[:, kc, ncol * 512 : (ncol + 1) * 512],
                                start=(kc == 0),
                                stop=(kc == 1),
                            )
                    ob = outpool.tile([128, D], bf16, tag="ob")
                    nc.vector.tensor_copy(out=ob[:], in_=ps[:])
                    nc.sync.dma_start(out[si * 128 : (si + 1) * 128, :], ob[:])

            rep_stack.close()

    nc.finalize()
    return nc


def _host_inputs(x, Wqkv, bqkv, qg, kg, Wproj, cos, sin):
    """Build the 8 per-core input maps (numpy, host-side sharding/layout)."""
    x = np.asarray(x, dtype=np.float32)
    Wqkv = np.asarray(Wqkv, dtype=np.float32)
    bqkv = np.asarray(bqkv, dtype=np.float32)
    qg = np.asarray(qg, dtype=np.float32)
    kg = np.asarray(kg, dtype=np.float32)
    Wproj = np.asarray(Wproj, dtype=np.float32)
    cos = np.asarray(cos, dtype=np.float32)
    sin = np.asarray(sin, dtype=np.float32)

    cosT2 = np.concatenate([cos.T, cos.T], axis=0).astype(BF16)  # [128, S]
    sf = np.concatenate([-sin[:, : HD // 2], sin[:, HD // 2 :]], axis=1)
    sinT2 = np.concatenate([sf.T, sf.T], axis=0).astype(BF16)  # [128, S]

    permm = np.zeros((128, 128), dtype=BF16)
    for mcol in range(128):
        rot = (mcol + 32) % 64 + 64 * (mcol // 64)
        permm[rot, mcol] = 1.0

    # Two group-members' mask matmuls ACCUMULATE into one [97,512] psum;
    # their column sets must be disjoint (see v3 notes).
    mask97 = np.zeros((128, 2, 97), dtype=BF16)
    mask97[0:64, 0, :] = 1.0
    mask97[0:64, 0, 32] = 0.0
    mask97[64:128, 0, 32] = 1.0
    mask97[:, 0, 64] = 0.0
    mask97[:, 0, 96] = 0.0
    mask97[0:64, 1, 64] = 1.0
    mask97[64:128, 1, 96] = 1.0

    # sel97[:, patt, :]: out rows 0:64 <- src row 64*patt, rows 64:128 <- row
    # 64*patt+32
    sel97 = np.zeros((97, 2, 128), dtype=BF16)
    sel97[0, 0, 0:64] = 1.0
    sel97[32, 0, 64:128] = 1.0
    sel97[64, 1, 0:64] = 1.0
    sel97[96, 1, 64:128] = 1.0

    qg4 = np.tile(qg, HPC)  # [256]
    kg4 = np.tile(kg, HPC)

    xT_b = [np.ascontiguousarray(x[b].T).astype(BF16) for b in range(B)]

    in_maps = []
    for core in range(N_CORES):
        b = core // 4
        hg = core % 4
        cq0 = hg * CW

        wqk_ = np.empty((D, 2 * CW), dtype=np.float32)
        wqk_[:, 0:CW] = Wqkv[:, cq0 : cq0 + CW] * qg4[None, :]
        wqk_[:, CW:] = Wqkv[:, D + cq0 : D + cq0 + CW] * kg4[None, :]

        wv_ = Wqkv[:, 2 * D + cq0 : 2 * D + cq0 + CW]

        bqk_ = np.zeros((128, 4), dtype=np.float32)
        bqkv_qk = np.concatenate(
            [bqkv[cq0 : cq0 + CW] * qg4, bqkv[D + cq0 : D + cq0 + CW] * kg4]
        )
        for m in range(4):
            bqk_[:, m] = bqkv_qk[m * 128 : (m + 1) * 128]

        bvb_ = np.broadcast_to(
            bqkv[2 * D + cq0 : 2 * D + cq0 + CW].astype(BF16)[None, :], (128, CW)
        ).copy()

        in_maps.append(
            {
                "xT": xT_b[b],
                "wqk": wqk_.astype(BF16),
                "wv": wv_.astype(BF16),
                "wpr": np.ascontiguousarray(Wproj[cq0 : cq0 + CW, :]).astype(BF16),
                "cosT2": cosT2,
                "sinT2": sinT2,
                "perm": permm,
                "mask97": mask97.reshape(128, 2 * 97),
                "sel97": sel97.reshape(97, 2 * 128),
                "bqk": bqk_,
                "bvb": bvb_,
            }
        )
    return in_maps


_NC_CACHE = None


def kernel(x, Wqkv, bqkv, qg, kg, Wproj, bproj, cos, sin):
    global LAST_RESULTS, _NC_CACHE
    from concourse.bass_utils import run_bass_kernel_spmd

    if _NC_CACHE is None:
        _NC_CACHE = _build_bass()
    nc = _NC_CACHE

    in_maps = _host_inputs(x, Wqkv, bqkv, qg, kg, Wproj, cos, sin)
    res = run_bass_kernel_spmd(nc, in_maps, core_ids=list(range(N_CORES)))
    LAST_RESULTS = res

    bproj = np.asarray(bproj, dtype=np.float32)
    out = np.zeros((B, S, D), dtype=np.float32)
    for b in range(B):
        acc = np.zeros((S, D), dtype=np.float32)
        for i in range(4):
            acc += res.results[4 * b + i]["out"].astype(np.float32)
        out[b] = acc + bproj[None, :]
    return out
